# revision 38
# baseline (speedup 1.0000x reference)
"""Causal self-attention on 8 trn2 NeuronCores.

Sharding: core = 2*b + g  (b in 0..3 batches, g in 0..1 head-groups of 8
heads). Each core computes, for its batch b and its 8 heads:
  qkv^T = Wqkv_slice^T @ x^T   (x^T pre-transposed on host)
  per-head causal softmax attention in scores^T layout:
   - score matmuls for the two heads of a pair run concurrently in
     disjoint PE row-groups (K=64 each, auto tile_position)
   - causal masking: gpsimd affine_select zeroes the 128-wide diagonal
     triangle of exp(scores); the fully-masked region is simply never
     read (the PV matmul starts ragged at the diagonal)
   - V is augmented with 8 ones-columns so the PV matmul accumulates the
     softmax denominator on psum partitions 64-71 for free
   - numerator/denominator are staged to SBUF immediately so the PSUM
     accumulator frees without waiting for the reciprocal chain
  partial out^T = y^T-normalized @ Wp_slice  -> [1024, 2048] bf16
Host gathers: out[b] = (partial[2b] + partial[2b+1]).T + b_proj.

Scheduling: QKV projection for head-pair p+1 is interleaved into the
attention pair-iterations of head-pair p, and the output projection for
head-pairs 0..2 fills head-pair 3's attention, so the PE never idles
during softmax and HAM stays un-throttled.
"""

import numpy as np
import ml_dtypes

B, T, E, H = 4, 2048, 1024, 16
HD = E // H  # 64
NEG = -30000.0

_CACHE = {}


def _build():
    from contextlib import ExitStack

    import concourse.bass as bass
    import concourse.mybir as mybir
    import concourse.tile as tile
    from concourse import bacc
    from concourse.masks import make_identity

    F32 = mybir.dt.float32
    BF16 = mybir.dt.bfloat16
    AF = mybir.ActivationFunctionType
    MUL = mybir.AluOpType.mult
    ADD = mybir.AluOpType.add

    nc = bacc.Bacc("TRN2", target_bir_lowering=False)
    xT = nc.dram_tensor("xT", [128, 8, T], BF16, kind="ExternalInput")
    wqkv = nc.dram_tensor("wqkv", [128, 8, 1536], BF16, kind="ExternalInput")
    bqkv = nc.dram_tensor("bqkv", [128, 12], F32, kind="ExternalInput")
    wp = nc.dram_tensor("wp", [128, 4, 1024], BF16, kind="ExternalInput")
    outT = nc.dram_tensor("outT", [E, T], BF16, kind="ExternalOutput")

    with tile.TileContext(nc) as tc, ExitStack() as ctx:
        const = ctx.enter_context(tc.tile_pool(name="const", bufs=1))
        ident32 = const.tile([128, 128], F32, tag="ident32")
        make_identity(nc, ident32[:])
        identr = const.tile([128, 128], BF16, tag="identr")
        nc.vector.tensor_copy(identr[:], ident32[:])
        # stacked 64x64 identities at partition 0 and 64 (for v-transpose,
        # whose lhsT sits at partition base 0 or 64)
        id2f = const.tile([128, 64], F32, tag="id2f")
        nc.gpsimd.memset(id2f[:], 0.0)
        for off in (0, 64):
            nc.gpsimd.affine_select(
                out=id2f[:],
                in_=id2f[:],
                compare_op=mybir.AluOpType.not_equal,
                fill=1.0,
                base=-off,
                pattern=[[-1, 64]],
                channel_multiplier=1,
            )
        id2 = const.tile([128, 64], BF16, tag="id2")
        nc.vector.tensor_copy(id2[:], id2f[:])
        # additive causal triangle mask [128, 128]: 0 where c >= ch else NEG.
        # Accumulated into the diagonal 128-col window of the score PSUM;
        # exp() then zeroes the masked region. Columns left of the window
        # hold anti-causal garbage that the ragged PV matmul never reads.
        mjf = const.tile([128, 128], F32, tag="maskf", name="maskf")
        nc.gpsimd.memset(mjf[:], 0.0)
        nc.gpsimd.affine_select(
            out=mjf[:],
            in_=mjf[:],
            compare_op=mybir.AluOpType.is_ge,
            fill=NEG,
            base=0,
            pattern=[[1, 128]],
            channel_multiplier=-1,
        )
        mtri = const.tile([128, 128], BF16, tag="mask", name="mask")
        nc.vector.tensor_copy(mtri[:], mjf[:])
        biasT = const.tile([128, 12], F32, tag="biasT")
        nc.sync.dma_start(biasT[:], bqkv[:])

        big = ctx.enter_context(tc.tile_pool(name="big", bufs=1))
        xTs = big.tile([128, 8, T], BF16, tag="xTs")
        qkvT = big.tile([128, 12, T], BF16, tag="qkvT")
        yT = big.tile([128, 4, T], BF16, tag="yT")

        ps = ctx.enter_context(tc.tile_pool(name="ps", bufs=1, space="PSUM"))
        wq_pool = ctx.enter_context(tc.tile_pool(name="wqp", bufs=3))
        vaug_pool = ctx.enter_context(tc.tile_pool(name="vaugp", bufs=2))
        pt_pool = ctx.enter_context(tc.tile_pool(name="ptp", bufs=2))
        sm_pool = ctx.enter_context(tc.tile_pool(name="smp", bufs=3))
        ob_pool = ctx.enter_context(tc.tile_pool(name="obp", bufs=2))

        state = {"wqm": {}, "vaug": {}, "obA": {}}

        def emit_dma(m):
            wqm = wq_pool.tile([128, 8, 128], BF16, tag="wqm", name=f"wqm{m}")
            nc.sync.dma_start(wqm[:], wqkv[:, :, m * 128 : (m + 1) * 128])
            state["wqm"][m] = wqm

        def emit_mm(m, j):
            wqm = state["wqm"][m]
            pq = ps.tile([128, 512], F32, tag="pq", bufs=2, name=f"pq{m}_{j}")
            for k in range(8):
                nc.tensor.matmul(
                    pq[:],
                    wqm[:, k, :],
                    xTs[:, k, j * 512 : (j + 1) * 512],
                    start=(k == 0),
                    stop=(k == 7),
                )
            nc.vector.tensor_scalar_add(
                qkvT[:, m, j * 512 : (j + 1) * 512], pq[:], biasT[:, m : m + 1]
            )

        def emit_vtrans(p, s, half):
            # transpose v for 8 key blocks into vaug (key-major, 128-stride;
            # cols 64-127 stay 1.0 so the PV matmul replicates the softmax
            # denominator across psum partitions 64-127)
            vaug = state["vaug"][p]
            pv = ps.tile([128, 512], F32, tag="pq", bufs=2, name=f"pv{p}_{s}_{half}")
            for i in range(8):
                kb = half * 8 + i
                nc.tensor.matmul(
                    pv[:, i * 64 : (i + 1) * 64],
                    qkvT[64 * s : 64 * s + 64, 3 * p + 2, kb * 128 : (kb + 1) * 128],
                    id2[64 * s : 64 * s + 64, :],
                    start=True,
                    stop=True,
                    tile_position=(64 * s, 0),
                )
            nc.vector.tensor_copy(
                vaug[:, s, half * 8 : half * 8 + 8, 0:64],
                pv[:].rearrange("p (i c) -> p i c", i=8),
            )

        def emit_vaug_alloc(p):
            vaug = vaug_pool.tile([128, 2, 16, 72], BF16, tag="vaug", name=f"vaug{p}")
            nc.gpsimd.memset(vaug[:], 1.0)
            state["vaug"][p] = vaug

        def qkv_quanta(p):
            m0, m1, m2 = 3 * p, 3 * p + 1, 3 * p + 2
            yield ("dma", m0)
            for j in range(4):
                yield ("mm", m0, j)
            yield ("dma", m1)
            for j in range(4):
                yield ("mm", m1, j)
            yield ("dma", m2)
            yield ("mm", m2, 0)
            yield ("mm", m2, 1)
            yield ("vaug", p)
            yield ("vtrans", p, 0, 0)
            yield ("vtrans", p, 1, 0)
            yield ("mm", m2, 2)
            yield ("mm", m2, 3)
            yield ("vtrans", p, 0, 1)
            yield ("vtrans", p, 1, 1)

        def emit_proj_partial(m, n):
            # output-projection contribution of head-pairs 0..2 (yT ready
            # before p=3's attention) — PE filler for the last head-pair
            if n == 0:
                state["obA"][m] = ob_pool.tile(
                    [128, T], BF16, tag="obA", bufs=8, name=f"obA{m}"
                )
            pn = ps.tile([128, 512], F32, tag="pq", bufs=2, name=f"pa{m}_{n}")
            for k in range(3):
                nc.tensor.matmul(
                    pn[:],
                    state["wps"][:, k, m * 128 : (m + 1) * 128],
                    yT[:, k, n * 512 : (n + 1) * 512],
                    start=(k == 0),
                    stop=(k == 2),
                )
            nc.vector.tensor_copy(state["obA"][m][:, n * 512 : (n + 1) * 512], pn[:])

        def run_quantum(q):
            if q[0] == "dma":
                emit_dma(q[1])
            elif q[0] == "mm":
                emit_mm(q[1], q[2])
            elif q[0] == "vaug":
                emit_vaug_alloc(q[1])
            elif q[0] == "proj":
                emit_proj_partial(q[1], q[2])
            else:
                emit_vtrans(q[1], q[2], q[3])

        def attention(p, filler):
            vaug = state["vaug"][p]
            for qc in range(4):
                kmax = 4 * qc + 4
                ym = {}
                for s in range(2):
                    ym[s] = ps.tile(
                        [128, 512], F32, tag=f"ym{s}", bufs=1, name=f"ym{p}_{qc}_{s}"
                    )
                for t in range(kmax // 2):
                    sc = {}
                    pt = {}
                    for s in range(2):
                        sc[s] = ps.tile(
                            [128, 1024],
                            F32,
                            tag=f"sc{s}",
                            bufs=1,
                            name=f"sc{p}_{qc}_{t}_{s}",
                        )
                        qT = qkvT[64 * s : 64 * s + 64, 3 * p, qc * 512 : qc * 512 + 512]
                        kT = qkvT[64 * s : 64 * s + 64, 3 * p + 1, :]
                        for i in range(2):
                            kb = 2 * t + i
                            d = kb - 4 * qc
                            c0s = max(0, 128 * d)
                            nc.tensor.matmul(
                                sc[s][:, i * 512 + c0s : (i + 1) * 512],
                                kT[:, kb * 128 : (kb + 1) * 128],
                                qkvT[
                                    64 * s : 64 * s + 64,
                                    3 * p,
                                    qc * 512 + c0s : qc * 512 + 512,
                                ],
                                start=True,
                                stop=True,
                            )
                    # PE filler while the scalar engine runs exp
                    if filler:
                        run_quantum(filler.pop(0))
                        if qc == 3 and len(filler) > (kmax // 2 - t):
                            run_quantum(filler.pop(0))
                    for s in range(2):
                        pt[s] = pt_pool.tile(
                            [128, 1024],
                            BF16,
                            tag=f"pt{s}",
                            name=f"pt{p}_{qc}_{t}_{s}",
                        )
                        nc.scalar.activation(pt[s][:], sc[s][:], AF.Exp, scale=0.125)
                        for i in range(2):
                            kb = 2 * t + i
                            d = kb - 4 * qc
                            if d >= 0:
                                c0 = 128 * d
                                nc.gpsimd.affine_select(
                                    out=pt[s][:, i * 512 + c0 : i * 512 + c0 + 128],
                                    in_=pt[s][:, i * 512 + c0 : i * 512 + c0 + 128],
                                    compare_op=mybir.AluOpType.is_ge,
                                    fill=0.0,
                                    base=0,
                                    pattern=[[1, 128]],
                                    channel_multiplier=-1,
                                )
                    for s in range(2):
                        for i in range(2):
                            kb = 2 * t + i
                            c0 = max(0, 128 * (kb - 4 * qc))
                            nc.tensor.matmul(
                                ym[s][0:72, c0:512],
                                vaug[:, s, kb, :],
                                pt[s][:, i * 512 + c0 : (i + 1) * 512],
                                start=(kb == 0),
                                stop=(kb == kmax - 1),
                            )
                # stage numerator + denominator to SBUF right away so the ym
                # banks free without waiting for the reciprocal chain; both
                # heads' denominators share one reciprocal (its cost is
                # per-column on the DVE)
                ymS = {}
                den2 = sm_pool.tile([40, 512], F32, tag="den2", bufs=2, name=f"d{p}{qc}")
                for s in range(2):
                    ymS[s] = sm_pool.tile(
                        [64, 512], F32, tag=f"ymS{s}", bufs=2, name=f"ymS{p}{qc}{s}"
                    )
                    nc.vector.tensor_copy(ymS[s][:], ym[s][0:64, :])
                    nc.vector.tensor_copy(den2[32 * s : 32 * s + 8, :], ym[s][64:72, :])
                rec = sm_pool.tile([40, 512], F32, tag="rec", bufs=2, name=f"rec{p}{qc}")
                nc.vector.reciprocal(rec[:], den2[:])
                recB = sm_pool.tile([8, 512], F32, tag="recB", bufs=2, name=f"rb{p}{qc}")
                nc.vector.tensor_copy(recB[:], rec[32:40, :])
                for s in range(2):
                    dbc = sm_pool.tile(
                        [64, 512], F32, tag=f"dbc{s}", bufs=2, name=f"dbc{p}{qc}{s}"
                    )
                    nc.gpsimd.partition_broadcast(
                        dbc[:], rec[0:1, :] if s == 0 else recB[0:1, :]
                    )
                    nc.vector.tensor_tensor(
                        out=yT[64 * s : 64 * s + 64, p, qc * 512 : qc * 512 + 512],
                        in0=ymS[s][:],
                        in1=dbc[:],
                        op=MUL,
                    )

        # ---------------- main schedule ----------------
        # p=0 weights first so the first QKV matmul isn't stuck behind the
        # full x DMA; x arrives in query-slices so m=0 starts at ~25% loaded
        for m in range(3):
            emit_dma(m)
        for j in range(4):
            nc.sync.dma_start(
                xTs[:, :, j * 512 : (j + 1) * 512], xT[:, :, j * 512 : (j + 1) * 512]
            )
        wps = big.tile([128, 4, 1024], BF16, tag="wps")
        nc.sync.dma_start(wps[:], wp[:])
        # preload the exp activation table during the DMA lead-in
        wrm = sm_pool.tile([1, 8], F32, tag="wrm")
        nc.gpsimd.memset(wrm[:], 0.0)
        wro = sm_pool.tile([1, 8], F32, tag="wro")
        nc.scalar.activation(wro[:], wrm[:], AF.Exp, scale=1.0)
        state["wps"] = wps
        for q in qkv_quanta(0):
            if q[0] == "dma" and q[1] < 3:
                continue
            run_quantum(q)
        for p in range(4):
            if p < 3:
                filler = list(qkv_quanta(p + 1))
            else:
                filler = [("proj", m, n) for m in range(8) for n in range(4)]
            attention(p, filler)
            for q in filler:
                run_quantum(q)

        # ---------------- output projection (last head-pair + combine) ----
        for m in range(8):
            ob = ob_pool.tile([128, T], BF16, tag="ob", name=f"ob{m}")
            for n in range(4):
                pn = ps.tile([128, 512], F32, tag="pq", bufs=2, name=f"pc{m}_{n}")
                nc.tensor.matmul(
                    pn[:],
                    wps[:, 3, m * 128 : (m + 1) * 128],
                    yT[:, 3, n * 512 : (n + 1) * 512],
                    start=True,
                    stop=True,
                )
                nc.vector.tensor_tensor(
                    out=ob[:, n * 512 : (n + 1) * 512],
                    in0=pn[:],
                    in1=state["obA"][m][:, n * 512 : (n + 1) * 512],
                    op=ADD,
                )
            nc.sync.dma_start(outT[m * 128 : (m + 1) * 128, :], ob[:])

    nc.compile()
    return nc


def _get_nc():
    if "nc" not in _CACHE:
        _CACHE["nc"] = _build()
    return _CACHE["nc"]


def _prep_core_inputs(xTb, w_attn, b_attn, w_proj, g):
    cols = []
    for p in range(4):
        off = 512 * g + 128 * p
        cols += [
            w_attn[:, off : off + 128],
            w_attn[:, E + off : E + off + 128],
            w_attn[:, 2 * E + off : 2 * E + off + 128],
        ]
    wq = np.concatenate(cols, axis=1)  # [1024, 1536]
    wq = np.ascontiguousarray(
        wq.reshape(8, 128, 1536).transpose(1, 0, 2), dtype=np.float32
    )
    bcols = []
    for p in range(4):
        off = 512 * g + 128 * p
        bcols += [
            b_attn[off : off + 128],
            b_attn[E + off : E + off + 128],
            b_attn[2 * E + off : 2 * E + off + 128],
        ]
    bq = np.stack(bcols, axis=1).astype(np.float32)  # [128, 12]
    wpr = np.concatenate(
        [w_proj[512 * g + 128 * p : 512 * g + 128 * p + 128, :] for p in range(4)],
        axis=0,
    )  # [512, 1024]
    wpr = np.ascontiguousarray(
        wpr.reshape(4, 128, 1024).transpose(1, 0, 2), dtype=np.float32
    )
    return {
        "xT": xTb,
        "wqkv": wq.astype(ml_dtypes.bfloat16),
        "bqkv": np.ascontiguousarray(bq),
        "wp": wpr.astype(ml_dtypes.bfloat16),
    }


def kernel(x, w_attn, b_attn, w_proj, b_proj, _trace=False):
    from concourse.bass_utils import run_bass_kernel_spmd

    x = np.asarray(x, dtype=np.float32)
    w_attn = np.asarray(w_attn, dtype=np.float32)
    b_attn = np.asarray(b_attn, dtype=np.float32)
    w_proj = np.asarray(w_proj, dtype=np.float32)
    b_proj = np.asarray(b_proj, dtype=np.float32)

    nc = _get_nc()
    xTs = []
    for b in range(B):
        xTb = np.ascontiguousarray(x[b].T).astype(ml_dtypes.bfloat16)
        xTs.append(
            np.ascontiguousarray(xTb.reshape(8, 128, T).transpose(1, 0, 2))
        )
    in_maps = [
        _prep_core_inputs(xTs[core // 2], w_attn, b_attn, w_proj, core % 2)
        for core in range(8)
    ]
    res = run_bass_kernel_spmd(nc, in_maps, core_ids=list(range(8)), trace=_trace)
    _CACHE["last_results"] = res
    out = np.empty((B, T, E), dtype=np.float32)
    for b in range(B):
        acc = res.results[2 * b]["outT"].astype(np.float32) + res.results[
            2 * b + 1
        ]["outT"].astype(np.float32)
        out[b] = acc.T + b_proj[None, :]
    return out


# revision 39
# speedup vs baseline: 1.0031x; 1.0031x over previous
"""Causal self-attention on 8 trn2 NeuronCores.

Sharding: core = 2*b + g  (b in 0..3 batches, g in 0..1 head-groups of 8
heads). Each core computes, for its batch b and its 8 heads:
  qkv^T = Wqkv_slice^T @ x^T   (x^T pre-transposed on host)
  per-head causal softmax attention in scores^T layout:
   - score matmuls for the two heads of a pair run concurrently in
     disjoint PE row-groups (K=64 each, auto tile_position)
   - causal masking: gpsimd affine_select zeroes the 128-wide diagonal
     triangle of exp(scores); the fully-masked region is simply never
     read (the PV matmul starts ragged at the diagonal)
   - V is augmented with 8 ones-columns so the PV matmul accumulates the
     softmax denominator on psum partitions 64-71 for free
   - numerator/denominator are staged to SBUF immediately so the PSUM
     accumulator frees without waiting for the reciprocal chain
  partial out^T = y^T-normalized @ Wp_slice  -> [1024, 2048] bf16
Host gathers: out[b] = (partial[2b] + partial[2b+1]).T + b_proj.

Scheduling: QKV projection for head-pair p+1 is interleaved into the
attention pair-iterations of head-pair p, and the output projection for
head-pairs 0..2 fills head-pair 3's attention, so the PE never idles
during softmax and HAM stays un-throttled.
"""

import numpy as np
import ml_dtypes

B, T, E, H = 4, 2048, 1024, 16
HD = E // H  # 64
NEG = -30000.0

_CACHE = {}


def _build():
    from contextlib import ExitStack

    import concourse.bass as bass
    import concourse.mybir as mybir
    import concourse.tile as tile
    from concourse import bacc
    from concourse.masks import make_identity

    F32 = mybir.dt.float32
    BF16 = mybir.dt.bfloat16
    AF = mybir.ActivationFunctionType
    MUL = mybir.AluOpType.mult
    ADD = mybir.AluOpType.add

    nc = bacc.Bacc("TRN2", target_bir_lowering=False)
    xT = nc.dram_tensor("xT", [4, 128, 8, 512], BF16, kind="ExternalInput")
    wqkv = nc.dram_tensor("wqkv", [128, 8, 1536], BF16, kind="ExternalInput")
    bqkv = nc.dram_tensor("bqkv", [128, 12], F32, kind="ExternalInput")
    wp = nc.dram_tensor("wp", [128, 4, 1024], BF16, kind="ExternalInput")
    outT = nc.dram_tensor("outT", [E, T], BF16, kind="ExternalOutput")

    with tile.TileContext(nc) as tc, ExitStack() as ctx:
        const = ctx.enter_context(tc.tile_pool(name="const", bufs=1))
        ident32 = const.tile([128, 128], F32, tag="ident32")
        make_identity(nc, ident32[:])
        identr = const.tile([128, 128], BF16, tag="identr")
        nc.vector.tensor_copy(identr[:], ident32[:])
        # stacked 64x64 identities at partition 0 and 64 (for v-transpose,
        # whose lhsT sits at partition base 0 or 64)
        id2f = const.tile([128, 64], F32, tag="id2f")
        nc.gpsimd.memset(id2f[:], 0.0)
        for off in (0, 64):
            nc.gpsimd.affine_select(
                out=id2f[:],
                in_=id2f[:],
                compare_op=mybir.AluOpType.not_equal,
                fill=1.0,
                base=-off,
                pattern=[[-1, 64]],
                channel_multiplier=1,
            )
        id2 = const.tile([128, 64], BF16, tag="id2")
        nc.vector.tensor_copy(id2[:], id2f[:])
        # additive causal triangle mask [128, 128]: 0 where c >= ch else NEG.
        # Accumulated into the diagonal 128-col window of the score PSUM;
        # exp() then zeroes the masked region. Columns left of the window
        # hold anti-causal garbage that the ragged PV matmul never reads.
        mjf = const.tile([128, 128], F32, tag="maskf", name="maskf")
        nc.gpsimd.memset(mjf[:], 0.0)
        nc.gpsimd.affine_select(
            out=mjf[:],
            in_=mjf[:],
            compare_op=mybir.AluOpType.is_ge,
            fill=NEG,
            base=0,
            pattern=[[1, 128]],
            channel_multiplier=-1,
        )
        mtri = const.tile([128, 128], BF16, tag="mask", name="mask")
        nc.vector.tensor_copy(mtri[:], mjf[:])
        biasT = const.tile([128, 12], F32, tag="biasT")
        nc.sync.dma_start(biasT[:], bqkv[:])

        big = ctx.enter_context(tc.tile_pool(name="big", bufs=1))
        xTs = big.tile([128, 8, T], BF16, tag="xTs")
        qkvT = big.tile([128, 12, T], BF16, tag="qkvT")
        yT = big.tile([128, 4, T], BF16, tag="yT")

        ps = ctx.enter_context(tc.tile_pool(name="ps", bufs=1, space="PSUM"))
        wq_pool = ctx.enter_context(tc.tile_pool(name="wqp", bufs=3))
        vaug_pool = ctx.enter_context(tc.tile_pool(name="vaugp", bufs=2))
        pt_pool = ctx.enter_context(tc.tile_pool(name="ptp", bufs=2))
        sm_pool = ctx.enter_context(tc.tile_pool(name="smp", bufs=3))
        ob_pool = ctx.enter_context(tc.tile_pool(name="obp", bufs=2))

        state = {"wqm": {}, "vaug": {}, "obA": {}}

        def emit_dma(m):
            wqm = wq_pool.tile([128, 8, 128], BF16, tag="wqm", name=f"wqm{m}")
            nc.sync.dma_start(wqm[:], wqkv[:, :, m * 128 : (m + 1) * 128])
            state["wqm"][m] = wqm

        def emit_mm(m, j):
            wqm = state["wqm"][m]
            pq = ps.tile([128, 512], F32, tag="pq", bufs=2, name=f"pq{m}_{j}")
            for k in range(8):
                nc.tensor.matmul(
                    pq[:],
                    wqm[:, k, :],
                    xTs[:, k, j * 512 : (j + 1) * 512],
                    start=(k == 0),
                    stop=(k == 7),
                )
            nc.vector.tensor_scalar_add(
                qkvT[:, m, j * 512 : (j + 1) * 512], pq[:], biasT[:, m : m + 1]
            )

        def emit_vtrans(p, s, half):
            # transpose v for 8 key blocks into vaug (key-major, 128-stride;
            # cols 64-127 stay 1.0 so the PV matmul replicates the softmax
            # denominator across psum partitions 64-127)
            vaug = state["vaug"][p]
            pv = ps.tile([128, 512], F32, tag="pq", bufs=2, name=f"pv{p}_{s}_{half}")
            for i in range(8):
                kb = half * 8 + i
                nc.tensor.matmul(
                    pv[:, i * 64 : (i + 1) * 64],
                    qkvT[64 * s : 64 * s + 64, 3 * p + 2, kb * 128 : (kb + 1) * 128],
                    id2[64 * s : 64 * s + 64, :],
                    start=True,
                    stop=True,
                    tile_position=(64 * s, 0),
                )
            nc.vector.tensor_copy(
                vaug[:, s, half * 8 : half * 8 + 8, 0:64],
                pv[:].rearrange("p (i c) -> p i c", i=8),
            )

        def emit_vaug_alloc(p):
            vaug = vaug_pool.tile([128, 2, 16, 72], BF16, tag="vaug", name=f"vaug{p}")
            nc.gpsimd.memset(vaug[:], 1.0)
            state["vaug"][p] = vaug

        def qkv_quanta(p):
            m0, m1, m2 = 3 * p, 3 * p + 1, 3 * p + 2
            yield ("dma", m0)
            for j in range(4):
                yield ("mm", m0, j)
            yield ("dma", m1)
            for j in range(4):
                yield ("mm", m1, j)
            yield ("dma", m2)
            yield ("mm", m2, 0)
            yield ("mm", m2, 1)
            yield ("vaug", p)
            yield ("vtrans", p, 0, 0)
            yield ("vtrans", p, 1, 0)
            yield ("mm", m2, 2)
            yield ("mm", m2, 3)
            yield ("vtrans", p, 0, 1)
            yield ("vtrans", p, 1, 1)

        def emit_proj_partial(m, n):
            # output-projection contribution of head-pairs 0..2 (yT ready
            # before p=3's attention) — PE filler for the last head-pair
            if n == 0:
                state["obA"][m] = ob_pool.tile(
                    [128, T], BF16, tag="obA", bufs=8, name=f"obA{m}"
                )
            pn = ps.tile([128, 512], F32, tag="pq", bufs=2, name=f"pa{m}_{n}")
            for k in range(3):
                nc.tensor.matmul(
                    pn[:],
                    state["wps"][:, k, m * 128 : (m + 1) * 128],
                    yT[:, k, n * 512 : (n + 1) * 512],
                    start=(k == 0),
                    stop=(k == 2),
                )
            nc.vector.tensor_copy(state["obA"][m][:, n * 512 : (n + 1) * 512], pn[:])

        def run_quantum(q):
            if q[0] == "dma":
                emit_dma(q[1])
            elif q[0] == "mm":
                emit_mm(q[1], q[2])
            elif q[0] == "vaug":
                emit_vaug_alloc(q[1])
            elif q[0] == "proj":
                emit_proj_partial(q[1], q[2])
            else:
                emit_vtrans(q[1], q[2], q[3])

        def attention(p, filler):
            vaug = state["vaug"][p]
            for qc in range(4):
                kmax = 4 * qc + 4
                ym = {}
                for s in range(2):
                    ym[s] = ps.tile(
                        [128, 512], F32, tag=f"ym{s}", bufs=1, name=f"ym{p}_{qc}_{s}"
                    )
                for t in range(kmax // 2):
                    sc = {}
                    pt = {}
                    for s in range(2):
                        sc[s] = ps.tile(
                            [128, 1024],
                            F32,
                            tag=f"sc{s}",
                            bufs=1,
                            name=f"sc{p}_{qc}_{t}_{s}",
                        )
                        qT = qkvT[64 * s : 64 * s + 64, 3 * p, qc * 512 : qc * 512 + 512]
                        kT = qkvT[64 * s : 64 * s + 64, 3 * p + 1, :]
                        for i in range(2):
                            kb = 2 * t + i
                            d = kb - 4 * qc
                            c0s = max(0, 128 * d)
                            nc.tensor.matmul(
                                sc[s][:, i * 512 + c0s : (i + 1) * 512],
                                kT[:, kb * 128 : (kb + 1) * 128],
                                qkvT[
                                    64 * s : 64 * s + 64,
                                    3 * p,
                                    qc * 512 + c0s : qc * 512 + 512,
                                ],
                                start=True,
                                stop=True,
                            )
                    # PE filler while the scalar engine runs exp
                    if filler:
                        run_quantum(filler.pop(0))
                        if qc == 3 and len(filler) > (kmax // 2 - t):
                            run_quantum(filler.pop(0))
                    for s in range(2):
                        pt[s] = pt_pool.tile(
                            [128, 1024],
                            BF16,
                            tag=f"pt{s}",
                            name=f"pt{p}_{qc}_{t}_{s}",
                        )
                        nc.scalar.activation(pt[s][:], sc[s][:], AF.Exp, scale=0.125)
                        for i in range(2):
                            kb = 2 * t + i
                            d = kb - 4 * qc
                            if d >= 0:
                                c0 = 128 * d
                                nc.gpsimd.affine_select(
                                    out=pt[s][:, i * 512 + c0 : i * 512 + c0 + 128],
                                    in_=pt[s][:, i * 512 + c0 : i * 512 + c0 + 128],
                                    compare_op=mybir.AluOpType.is_ge,
                                    fill=0.0,
                                    base=0,
                                    pattern=[[1, 128]],
                                    channel_multiplier=-1,
                                )
                    for s in range(2):
                        for i in range(2):
                            kb = 2 * t + i
                            c0 = max(0, 128 * (kb - 4 * qc))
                            nc.tensor.matmul(
                                ym[s][0:72, c0:512],
                                vaug[:, s, kb, :],
                                pt[s][:, i * 512 + c0 : (i + 1) * 512],
                                start=(kb == 0),
                                stop=(kb == kmax - 1),
                            )
                # stage numerator + denominator to SBUF right away so the ym
                # banks free without waiting for the reciprocal chain; both
                # heads' denominators share one reciprocal (its cost is
                # per-column on the DVE)
                ymS = {}
                den2 = sm_pool.tile([40, 512], F32, tag="den2", bufs=2, name=f"d{p}{qc}")
                for s in range(2):
                    ymS[s] = sm_pool.tile(
                        [64, 512], F32, tag=f"ymS{s}", bufs=2, name=f"ymS{p}{qc}{s}"
                    )
                    nc.vector.tensor_copy(ymS[s][:], ym[s][0:64, :])
                    nc.vector.tensor_copy(den2[32 * s : 32 * s + 8, :], ym[s][64:72, :])
                rec = sm_pool.tile([40, 512], F32, tag="rec", bufs=2, name=f"rec{p}{qc}")
                nc.vector.reciprocal(rec[:], den2[:])
                recB = sm_pool.tile([8, 512], F32, tag="recB", bufs=2, name=f"rb{p}{qc}")
                nc.vector.tensor_copy(recB[:], rec[32:40, :])
                for s in range(2):
                    dbc = sm_pool.tile(
                        [64, 512], F32, tag=f"dbc{s}", bufs=2, name=f"dbc{p}{qc}{s}"
                    )
                    nc.gpsimd.partition_broadcast(
                        dbc[:], rec[0:1, :] if s == 0 else recB[0:1, :]
                    )
                    nc.vector.tensor_tensor(
                        out=yT[64 * s : 64 * s + 64, p, qc * 512 : qc * 512 + 512],
                        in0=ymS[s][:],
                        in1=dbc[:],
                        op=MUL,
                    )

        # ---------------- main schedule ----------------
        # p=0 weights first so the first QKV matmul isn't stuck behind the
        # full x DMA; x arrives in query-slices so m=0 starts at ~25% loaded
        for m in range(3):
            emit_dma(m)
        for j in range(4):
            nc.sync.dma_start(xTs[:, :, j * 512 : (j + 1) * 512], xT[j])
        wps = big.tile([128, 4, 1024], BF16, tag="wps")
        nc.sync.dma_start(wps[:], wp[:])
        # preload the exp activation table during the DMA lead-in
        wrm = sm_pool.tile([1, 8], F32, tag="wrm")
        nc.gpsimd.memset(wrm[:], 0.0)
        wro = sm_pool.tile([1, 8], F32, tag="wro")
        nc.scalar.activation(wro[:], wrm[:], AF.Exp, scale=1.0)
        state["wps"] = wps
        for q in qkv_quanta(0):
            if q[0] == "dma" and q[1] < 3:
                continue
            run_quantum(q)
        for p in range(4):
            if p < 3:
                filler = list(qkv_quanta(p + 1))
            else:
                filler = [("proj", m, n) for m in range(8) for n in range(4)]
            attention(p, filler)
            for q in filler:
                run_quantum(q)

        # ---------------- output projection (last head-pair + combine) ----
        for m in range(8):
            ob = ob_pool.tile([128, T], BF16, tag="ob", name=f"ob{m}")
            for n in range(4):
                pn = ps.tile([128, 512], F32, tag="pq", bufs=2, name=f"pc{m}_{n}")
                nc.tensor.matmul(
                    pn[:],
                    wps[:, 3, m * 128 : (m + 1) * 128],
                    yT[:, 3, n * 512 : (n + 1) * 512],
                    start=True,
                    stop=True,
                )
                nc.vector.tensor_tensor(
                    out=ob[:, n * 512 : (n + 1) * 512],
                    in0=pn[:],
                    in1=state["obA"][m][:, n * 512 : (n + 1) * 512],
                    op=ADD,
                )
            nc.sync.dma_start(outT[m * 128 : (m + 1) * 128, :], ob[:])

    nc.compile()
    return nc


def _get_nc():
    if "nc" not in _CACHE:
        _CACHE["nc"] = _build()
    return _CACHE["nc"]


def _prep_core_inputs(xTb, w_attn, b_attn, w_proj, g):
    cols = []
    for p in range(4):
        off = 512 * g + 128 * p
        cols += [
            w_attn[:, off : off + 128],
            w_attn[:, E + off : E + off + 128],
            w_attn[:, 2 * E + off : 2 * E + off + 128],
        ]
    wq = np.concatenate(cols, axis=1)  # [1024, 1536]
    wq = np.ascontiguousarray(
        wq.reshape(8, 128, 1536).transpose(1, 0, 2), dtype=np.float32
    )
    bcols = []
    for p in range(4):
        off = 512 * g + 128 * p
        bcols += [
            b_attn[off : off + 128],
            b_attn[E + off : E + off + 128],
            b_attn[2 * E + off : 2 * E + off + 128],
        ]
    bq = np.stack(bcols, axis=1).astype(np.float32)  # [128, 12]
    wpr = np.concatenate(
        [w_proj[512 * g + 128 * p : 512 * g + 128 * p + 128, :] for p in range(4)],
        axis=0,
    )  # [512, 1024]
    wpr = np.ascontiguousarray(
        wpr.reshape(4, 128, 1024).transpose(1, 0, 2), dtype=np.float32
    )
    return {
        "xT": xTb,
        "wqkv": wq.astype(ml_dtypes.bfloat16),
        "bqkv": np.ascontiguousarray(bq),
        "wp": wpr.astype(ml_dtypes.bfloat16),
    }


def kernel(x, w_attn, b_attn, w_proj, b_proj, _trace=False):
    from concourse.bass_utils import run_bass_kernel_spmd

    x = np.asarray(x, dtype=np.float32)
    w_attn = np.asarray(w_attn, dtype=np.float32)
    b_attn = np.asarray(b_attn, dtype=np.float32)
    w_proj = np.asarray(w_proj, dtype=np.float32)
    b_proj = np.asarray(b_proj, dtype=np.float32)

    nc = _get_nc()
    xTs = []
    for b in range(B):
        xTb = np.ascontiguousarray(x[b].T).astype(ml_dtypes.bfloat16)
        xTs.append(
            np.ascontiguousarray(xTb.reshape(8, 128, 4, 512).transpose(2, 1, 0, 3))
        )
    in_maps = [
        _prep_core_inputs(xTs[core // 2], w_attn, b_attn, w_proj, core % 2)
        for core in range(8)
    ]
    res = run_bass_kernel_spmd(nc, in_maps, core_ids=list(range(8)), trace=_trace)
    _CACHE["last_results"] = res
    out = np.empty((B, T, E), dtype=np.float32)
    for b in range(B):
        acc = res.results[2 * b]["outT"].astype(np.float32) + res.results[
            2 * b + 1
        ]["outT"].astype(np.float32)
        out[b] = acc.T + b_proj[None, :]
    return out


# revision 45
# speedup vs baseline: 1.0320x; 1.0289x over previous
"""Causal self-attention on 8 trn2 NeuronCores.

Sharding: core = 2*b + g  (b in 0..3 batches, g in 0..1 head-groups of 8
heads). Each core computes, for its batch b and its 8 heads:
  qkv^T = Wqkv_slice^T @ x^T   (x^T pre-transposed on host)
  per-head causal softmax attention in scores^T layout:
   - score matmuls for the two heads of a pair run concurrently in
     disjoint PE row-groups (K=64 each, auto tile_position)
   - causal masking: gpsimd affine_select zeroes the 128-wide diagonal
     triangle of exp(scores); the fully-masked region is simply never
     read (the PV matmul starts ragged at the diagonal)
   - V is augmented with 8 ones-columns so the PV matmul accumulates the
     softmax denominator on psum partitions 64-71 for free
   - numerator/denominator are staged to SBUF immediately so the PSUM
     accumulator frees without waiting for the reciprocal chain
  partial out^T = y^T-normalized @ Wp_slice  -> [1024, 2048] bf16
Host gathers: out[b] = (partial[2b] + partial[2b+1]).T + b_proj.

Scheduling: QKV projection for head-pair p+1 is interleaved into the
attention pair-iterations of head-pair p, and the output projection for
head-pairs 0..2 fills head-pair 3's attention, so the PE never idles
during softmax and HAM stays un-throttled.
"""

import numpy as np
import ml_dtypes

B, T, E, H = 4, 2048, 1024, 16
HD = E // H  # 64
NEG = -30000.0

_CACHE = {}


def _build():
    from contextlib import ExitStack

    import concourse.bass as bass
    import concourse.mybir as mybir
    import concourse.tile as tile
    from concourse import bacc
    from concourse.masks import make_identity

    F32 = mybir.dt.float32
    BF16 = mybir.dt.bfloat16
    AF = mybir.ActivationFunctionType
    MUL = mybir.AluOpType.mult
    ADD = mybir.AluOpType.add

    nc = bacc.Bacc("TRN2", target_bir_lowering=False)
    xT = nc.dram_tensor("xT", [4, 128, 8, 512], BF16, kind="ExternalInput")
    wqkv = nc.dram_tensor("wqkv", [12, 128, 8, 128], BF16, kind="ExternalInput")
    bqkv = nc.dram_tensor("bqkv", [128, 12], F32, kind="ExternalInput")
    wp = nc.dram_tensor("wp", [128, 4, 1024], BF16, kind="ExternalInput")
    outT = nc.dram_tensor("outT", [E, T], BF16, kind="ExternalOutput")

    with tile.TileContext(nc) as tc, ExitStack() as ctx:
        const = ctx.enter_context(tc.tile_pool(name="const", bufs=1))
        ident32 = const.tile([128, 128], F32, tag="ident32")
        make_identity(nc, ident32[:])
        identr = const.tile([128, 128], BF16, tag="identr")
        nc.vector.tensor_copy(identr[:], ident32[:])
        # stacked 64x64 identities at partition 0 and 64 (for v-transpose,
        # whose lhsT sits at partition base 0 or 64)
        id2f = const.tile([128, 64], F32, tag="id2f")
        nc.gpsimd.memset(id2f[:], 0.0)
        for off in (0, 64):
            nc.gpsimd.affine_select(
                out=id2f[:],
                in_=id2f[:],
                compare_op=mybir.AluOpType.not_equal,
                fill=1.0,
                base=-off,
                pattern=[[-1, 64]],
                channel_multiplier=1,
            )
        id2 = const.tile([128, 64], BF16, tag="id2")
        nc.vector.tensor_copy(id2[:], id2f[:])
        # additive causal triangle mask [128, 128]: 0 where c >= ch else NEG.
        # Accumulated into the diagonal 128-col window of the score PSUM;
        # exp() then zeroes the masked region. Columns left of the window
        # hold anti-causal garbage that the ragged PV matmul never reads.
        mjf = const.tile([128, 128], F32, tag="maskf", name="maskf")
        nc.gpsimd.memset(mjf[:], 0.0)
        nc.gpsimd.affine_select(
            out=mjf[:],
            in_=mjf[:],
            compare_op=mybir.AluOpType.is_ge,
            fill=NEG,
            base=0,
            pattern=[[1, 128]],
            channel_multiplier=-1,
        )
        mtri = const.tile([128, 128], BF16, tag="mask", name="mask")
        nc.vector.tensor_copy(mtri[:], mjf[:])
        biasT = const.tile([128, 12], F32, tag="biasT")
        nc.sync.dma_start(biasT[:], bqkv[:])

        big = ctx.enter_context(tc.tile_pool(name="big", bufs=1))
        xTs = big.tile([128, 8, T], BF16, tag="xTs")
        qkvT = big.tile([128, 12, T], BF16, tag="qkvT")
        yT = big.tile([128, 4, T], BF16, tag="yT")

        ps = ctx.enter_context(tc.tile_pool(name="ps", bufs=1, space="PSUM"))
        wq_pool = ctx.enter_context(tc.tile_pool(name="wqp", bufs=3))
        vaug_pool = ctx.enter_context(tc.tile_pool(name="vaugp", bufs=2))
        pt_pool = ctx.enter_context(tc.tile_pool(name="ptp", bufs=2))
        sm_pool = ctx.enter_context(tc.tile_pool(name="smp", bufs=3))
        ob_pool = ctx.enter_context(tc.tile_pool(name="obp", bufs=2))

        state = {"wqm": {}, "vaug": {}, "obA": {}}

        def emit_dma(m):
            wqm = wq_pool.tile([128, 8, 128], BF16, tag="wqm", name=f"wqm{m}")
            nc.sync.dma_start(wqm[:], wqkv[m])
            state["wqm"][m] = wqm

        def emit_mm(m, j):
            wqm = state["wqm"][m]
            pq = ps.tile([128, 512], F32, tag="pq", bufs=2, name=f"pq{m}_{j}")
            for k in range(8):
                nc.tensor.matmul(
                    pq[:],
                    wqm[:, k, :],
                    xTs[:, k, j * 512 : (j + 1) * 512],
                    start=(k == 0),
                    stop=(k == 7),
                )
            nc.vector.tensor_scalar_add(
                qkvT[:, m, j * 512 : (j + 1) * 512], pq[:], biasT[:, m : m + 1]
            )

        def emit_vtrans(p, half):
            # transpose v for 8 key blocks of both heads into vaug
            # (key-major; cols 64-71 stay 1.0 for the denominator trick);
            # the two heads alternate PE row-groups so weight loads overlap
            vaug = state["vaug"][p]
            pv = {}
            for s in range(2):
                pv[s] = ps.tile(
                    [128, 512], F32, tag="pq", bufs=2, name=f"pv{p}_{s}_{half}"
                )
            for i in range(8):
                kb = half * 8 + i
                for s in range(2):
                    nc.tensor.matmul(
                        pv[s][:, i * 64 : (i + 1) * 64],
                        qkvT[64 * s : 64 * s + 64, 3 * p + 2, kb * 128 : (kb + 1) * 128],
                        id2[64 * s : 64 * s + 64, :],
                        start=True,
                        stop=True,
                        tile_position=(64 * s, 0),
                    )
            for s in range(2):
                nc.vector.tensor_copy(
                    vaug[:, s, half * 8 : half * 8 + 8, 0:64],
                    pv[s][:].rearrange("p (i c) -> p i c", i=8),
                )

        def emit_vaug_alloc(p):
            vaug = vaug_pool.tile([128, 2, 16, 72], BF16, tag="vaug", name=f"vaug{p}")
            nc.gpsimd.memset(vaug[:], 1.0)
            state["vaug"][p] = vaug

        def qkv_quanta(p):
            m0, m1, m2 = 3 * p, 3 * p + 1, 3 * p + 2
            yield ("dma", m0)
            for j in range(4):
                yield ("mm", m0, j)
            yield ("dma", m1)
            for j in range(4):
                yield ("mm", m1, j)
            yield ("dma", m2)
            yield ("mm", m2, 0)
            yield ("mm", m2, 1)
            yield ("vaug", p)
            yield ("vtrans", p, 0)
            yield ("mm", m2, 2)
            yield ("mm", m2, 3)
            yield ("vtrans", p, 1)

        def emit_proj_partial(m, n):
            # output-projection contribution of head-pairs 0..2 (yT ready
            # before p=3's attention) — PE filler for the last head-pair
            if n == 0:
                state["obA"][m] = ob_pool.tile(
                    [128, T], BF16, tag="obA", bufs=8, name=f"obA{m}"
                )
            pn = ps.tile([128, 512], F32, tag="pq", bufs=2, name=f"pa{m}_{n}")
            for k in range(3):
                nc.tensor.matmul(
                    pn[:],
                    state["wps"][:, k, m * 128 : (m + 1) * 128],
                    yT[:, k, n * 512 : (n + 1) * 512],
                    start=(k == 0),
                    stop=(k == 2),
                )
            nc.vector.tensor_copy(state["obA"][m][:, n * 512 : (n + 1) * 512], pn[:])

        def run_quantum(q):
            if q[0] == "dma":
                emit_dma(q[1])
            elif q[0] == "mm":
                emit_mm(q[1], q[2])
            elif q[0] == "vaug":
                emit_vaug_alloc(q[1])
            elif q[0] == "proj":
                emit_proj_partial(q[1], q[2])
            else:
                emit_vtrans(q[1], q[2])

        def attention(p, filler):
            vaug = state["vaug"][p]
            for qc in range(4):
                kmax = 4 * qc + 4
                ym = {}
                for s in range(2):
                    ym[s] = ps.tile(
                        [128, 512], F32, tag=f"ym{s}", bufs=1, name=f"ym{p}_{qc}_{s}"
                    )
                for t in range(kmax // 2):
                    sc = {}
                    pt = {}
                    for s in range(2):
                        sc[s] = ps.tile(
                            [128, 1024],
                            F32,
                            tag=f"sc{s}",
                            bufs=1,
                            name=f"sc{p}_{qc}_{t}_{s}",
                        )
                    for i in range(2):
                        for s in range(2):
                            kb = 2 * t + i
                            d = kb - 4 * qc
                            c0s = max(0, 128 * d)
                            nc.tensor.matmul(
                                sc[s][:, i * 512 + c0s : (i + 1) * 512],
                                qkvT[
                                    64 * s : 64 * s + 64,
                                    3 * p + 1,
                                    kb * 128 : (kb + 1) * 128,
                                ],
                                qkvT[
                                    64 * s : 64 * s + 64,
                                    3 * p,
                                    qc * 512 + c0s : qc * 512 + 512,
                                ],
                                start=True,
                                stop=True,
                            )
                    # PE filler while the scalar engine runs exp
                    if filler and not (t == 0 and qc > 0):
                        run_quantum(filler.pop(0))
                        if qc == 3 and len(filler) > (kmax // 2 - t):
                            run_quantum(filler.pop(0))
                    for s in range(2):
                        pt[s] = pt_pool.tile(
                            [128, 1024],
                            BF16,
                            tag=f"pt{s}",
                            name=f"pt{p}_{qc}_{t}_{s}",
                        )
                        nc.scalar.activation(pt[s][:], sc[s][:], AF.Exp, scale=0.125)
                        for i in range(2):
                            kb = 2 * t + i
                            d = kb - 4 * qc
                            if d >= 0:
                                c0 = 128 * d
                                nc.gpsimd.affine_select(
                                    out=pt[s][:, i * 512 + c0 : i * 512 + c0 + 128],
                                    in_=pt[s][:, i * 512 + c0 : i * 512 + c0 + 128],
                                    compare_op=mybir.AluOpType.is_ge,
                                    fill=0.0,
                                    base=0,
                                    pattern=[[1, 128]],
                                    channel_multiplier=-1,
                                )
                    for s in range(2):
                        for i in range(2):
                            kb = 2 * t + i
                            c0 = max(0, 128 * (kb - 4 * qc))
                            nc.tensor.matmul(
                                ym[s][0:72, c0:512],
                                vaug[:, s, kb, :],
                                pt[s][:, i * 512 + c0 : (i + 1) * 512],
                                start=(kb == 0),
                                stop=(kb == kmax - 1),
                            )
                if filler:
                    run_quantum(filler.pop(0))
                # stage numerator + denominator to SBUF right away so the ym
                # banks free without waiting for the reciprocal chain; both
                # heads' denominators share one reciprocal (its cost is
                # per-column on the DVE)
                ymS = {}
                den2 = sm_pool.tile([40, 512], F32, tag="den2", bufs=2, name=f"d{p}{qc}")
                for s in range(2):
                    ymS[s] = sm_pool.tile(
                        [64, 512], F32, tag=f"ymS{s}", bufs=2, name=f"ymS{p}{qc}{s}"
                    )
                    nc.vector.tensor_copy(ymS[s][:], ym[s][0:64, :])
                    nc.vector.tensor_copy(den2[32 * s : 32 * s + 8, :], ym[s][64:72, :])
                rec = sm_pool.tile([40, 512], F32, tag="rec", bufs=2, name=f"rec{p}{qc}")
                nc.vector.reciprocal(rec[:], den2[:])
                recB = sm_pool.tile([8, 512], F32, tag="recB", bufs=2, name=f"rb{p}{qc}")
                nc.vector.tensor_copy(recB[:], rec[32:40, :])
                for s in range(2):
                    dbc = sm_pool.tile(
                        [64, 512], F32, tag=f"dbc{s}", bufs=2, name=f"dbc{p}{qc}{s}"
                    )
                    nc.gpsimd.partition_broadcast(
                        dbc[:], rec[0:1, :] if s == 0 else recB[0:1, :]
                    )
                    nc.vector.tensor_tensor(
                        out=yT[64 * s : 64 * s + 64, p, qc * 512 : qc * 512 + 512],
                        in0=ymS[s][:],
                        in1=dbc[:],
                        op=MUL,
                    )

        # ---------------- main schedule ----------------
        # p=0 weights first so the first QKV matmul isn't stuck behind the
        # full x DMA; x arrives in query-slices so m=0 starts at ~25% loaded
        for m in range(3):
            emit_dma(m)
        for j in range(4):
            nc.sync.dma_start(xTs[:, :, j * 512 : (j + 1) * 512], xT[j])
        wps = big.tile([128, 4, 1024], BF16, tag="wps")
        nc.sync.dma_start(wps[:], wp[:])
        # preload the exp activation table during the DMA lead-in
        wrm = sm_pool.tile([1, 8], F32, tag="wrm")
        nc.gpsimd.memset(wrm[:], 0.0)
        wro = sm_pool.tile([1, 8], F32, tag="wro")
        nc.scalar.activation(wro[:], wrm[:], AF.Exp, scale=1.0)
        state["wps"] = wps
        for q in qkv_quanta(0):
            if q[0] == "dma" and q[1] < 3:
                continue
            run_quantum(q)
        for p in range(4):
            if p < 3:
                filler = list(qkv_quanta(p + 1))
            else:
                filler = [("proj", m, n) for m in range(8) for n in range(4)]
            attention(p, filler)
            for q in filler:
                run_quantum(q)

        # ---------------- output projection (last head-pair + combine) ----
        for m in range(8):
            ob = ob_pool.tile([128, T], BF16, tag="ob", name=f"ob{m}")
            for n in range(4):
                pn = ps.tile([128, 512], F32, tag="pq", bufs=2, name=f"pc{m}_{n}")
                nc.tensor.matmul(
                    pn[:],
                    wps[:, 3, m * 128 : (m + 1) * 128],
                    yT[:, 3, n * 512 : (n + 1) * 512],
                    start=True,
                    stop=True,
                )
                nc.vector.tensor_tensor(
                    out=ob[:, n * 512 : (n + 1) * 512],
                    in0=pn[:],
                    in1=state["obA"][m][:, n * 512 : (n + 1) * 512],
                    op=ADD,
                )
            nc.sync.dma_start(outT[m * 128 : (m + 1) * 128, :], ob[:])

    nc.compile()
    return nc


def _get_nc():
    if "nc" not in _CACHE:
        _CACHE["nc"] = _build()
    return _CACHE["nc"]


def _prep_core_inputs(xTb, w_attn, b_attn, w_proj, g):
    cols = []
    for p in range(4):
        off = 512 * g + 128 * p
        cols += [
            w_attn[:, off : off + 128],
            w_attn[:, E + off : E + off + 128],
            w_attn[:, 2 * E + off : 2 * E + off + 128],
        ]
    wq = np.concatenate(cols, axis=1)  # [1024, 1536]
    wq = np.ascontiguousarray(
        wq.reshape(8, 128, 12, 128).transpose(2, 1, 0, 3), dtype=np.float32
    )
    bcols = []
    for p in range(4):
        off = 512 * g + 128 * p
        bcols += [
            b_attn[off : off + 128],
            b_attn[E + off : E + off + 128],
            b_attn[2 * E + off : 2 * E + off + 128],
        ]
    bq = np.stack(bcols, axis=1).astype(np.float32)  # [128, 12]
    wpr = np.concatenate(
        [w_proj[512 * g + 128 * p : 512 * g + 128 * p + 128, :] for p in range(4)],
        axis=0,
    )  # [512, 1024]
    wpr = np.ascontiguousarray(
        wpr.reshape(4, 128, 1024).transpose(1, 0, 2), dtype=np.float32
    )
    return {
        "xT": xTb,
        "wqkv": wq.astype(ml_dtypes.bfloat16),
        "bqkv": np.ascontiguousarray(bq),
        "wp": wpr.astype(ml_dtypes.bfloat16),
    }


def kernel(x, w_attn, b_attn, w_proj, b_proj, _trace=False):
    from concourse.bass_utils import run_bass_kernel_spmd

    x = np.asarray(x, dtype=np.float32)
    w_attn = np.asarray(w_attn, dtype=np.float32)
    b_attn = np.asarray(b_attn, dtype=np.float32)
    w_proj = np.asarray(w_proj, dtype=np.float32)
    b_proj = np.asarray(b_proj, dtype=np.float32)

    nc = _get_nc()
    xTs = []
    for b in range(B):
        xTb = np.ascontiguousarray(x[b].T).astype(ml_dtypes.bfloat16)
        xTs.append(
            np.ascontiguousarray(xTb.reshape(8, 128, 4, 512).transpose(2, 1, 0, 3))
        )
    in_maps = [
        _prep_core_inputs(xTs[core // 2], w_attn, b_attn, w_proj, core % 2)
        for core in range(8)
    ]
    res = run_bass_kernel_spmd(nc, in_maps, core_ids=list(range(8)), trace=_trace)
    _CACHE["last_results"] = res
    out = np.empty((B, T, E), dtype=np.float32)
    for b in range(B):
        acc = res.results[2 * b]["outT"].astype(np.float32) + res.results[
            2 * b + 1
        ]["outT"].astype(np.float32)
        out[b] = acc.T + b_proj[None, :]
    return out
